# revision 15
# baseline (speedup 1.0000x reference)
"""AdaptiveVectorQuantizer eval-forward on 8 TRN2 NeuronCores.

Reference computation (see problem): for flattened input vectors x [N=65536, 64]
and codebook c [512, 64], compute d2(n, p) = ||x_n||^2 + ||c_p||^2 - 2 x_n.c_p,
then for 9 prefix levels k = 2, 4, ..., 512 take idx_a = argmin_p<k d2 and emit
quant_a = c[idx_a] reshaped to [B, D, H, W].  Output: ([9, 16, 64, 64, 64], c).

Strategy (data parallel, no collectives):
  - Core c owns batches {2c, 2c+1}: 8192 vectors, 64 tiles of 128.
  - argmin_p d2 == argmax_p (x.c - ||c||^2/2): fold the -||c||^2/2 term into an
    extra contraction row, so scores come from one [65,128]x[65,512] matmul per
    tile (PSUM [128 vec, 512 codes]).
  - Prefix argmax via dyadic segments [0:2],[2:4],[4:8],...,[256:512]: batched
    DVE reduce_max per segment, within-segment index via scalar_tensor_tensor
    (mask = (s >= segmax), out = mask*iota, accum-sum -> index), then a 9-step
    running select chain over levels.
  - Gather c[idx] with gpsimd.indirect_dma_start (per-partition int32 offsets,
    256B codebook rows) into [128, L, T, 64], then one line-rate output DMA per
    group (9KB contiguous per partition).  The host unshard de-stripes the
    [G, 128, L, T, 64] device layout back to [L, B, D, H, W].
"""

import numpy as np

B, D, HW = 16, 64, 4096
P = 512
L = 9
NCORES = 8
BPC = B // NCORES            # batches per core
NLOC = BPC * HW              # vectors per core
TILE = 128
TILES = NLOC // TILE         # 64
TPG = 16                     # tiles per group
GROUPS = TILES // TPG        # 4
SEGS = [(0, 2), (2, 4)] + [(1 << a, 1 << (a + 1)) for a in range(2, 9)]

_CACHE = {}


def _build_nc(reps=1):
    import concourse.bass as bass
    import concourse.bacc as bacc
    import concourse.mybir as mybir
    from concourse.tile import TileContext

    dt = mybir.dt
    Alu = mybir.AluOpType
    Ax = mybir.AxisListType

    nc = bacc.Bacc("TRN2", target_bir_lowering=False, debug=False,
                   num_devices=NCORES)

    x = nc.dram_tensor("x", [BPC, D, HW], dt.float32, kind="ExternalInput")
    cT = nc.dram_tensor("cT", [D + 1, P], dt.float32, kind="ExternalInput")
    cb = nc.dram_tensor("cb", [P, D], dt.float32, kind="ExternalInput")
    iota = nc.dram_tensor("iota", [128, P], dt.float32, kind="ExternalInput")
    xn2 = nc.dram_tensor("xn2", [128, TILES], dt.float32,
                         kind="ExternalInput")
    out = nc.dram_tensor("out", [GROUPS, 128, L * TPG * D], dt.float32,
                         kind="ExternalOutput")

    with TileContext(nc) as tc:
        with (
            tc.tile_pool(name="const", bufs=1) as const_pool,
            tc.tile_pool(name="s", bufs=2) as s_pool,
            tc.tile_pool(name="psum", bufs=8, space="PSUM") as psum_pool,
            tc.tile_pool(name="stats", bufs=2) as st_pool,
            tc.tile_pool(name="junk", bufs=1) as junk_pool,
            tc.tile_pool(name="g", bufs=2) as g_pool,
        ):
            # ---- constants / inputs ----
            cT_sb = const_pool.tile([D + 1, P], dt.float32)
            nc.sync.dma_start(out=cT_sb[:, :], in_=cT[:, :])
            iota_sb = const_pool.tile([128, P], dt.float32)
            nc.sync.dma_start(out=iota_sb[:, :], in_=iota[:, :])
            xn2_sb = const_pool.tile([128, TILES], dt.float32)
            nc.sync.dma_start(out=xn2_sb[:, :], in_=xn2[:, :])

            x_sb = const_pool.tile([D + 1, BPC, HW], dt.float32)
            for b in range(BPC):
                nc.sync.dma_start(out=x_sb[0:D, b, :], in_=x[b, :, :])
            nc.vector.memset(x_sb[D:D + 1, :, :], 1.0)

            junk = junk_pool.tile([128, P], dt.float32)

          # repeated executions of the whole pipeline (timing harness)

            for rep in range(reps):
             for g in range(GROUPS):
                s_sb = s_pool.tile([128, TPG, P], dt.float32, tag="s")
                for tl in range(TPG):
                    t = g * TPG + tl
                    b, off = divmod(t * TILE, HW)
                    ps = psum_pool.tile([128, P], dt.float32, tag="ps")
                    nc.tensor.matmul(ps[:, :], x_sb[:, b, off:off + TILE],
                                     cT_sb[:, :], start=True, stop=True)
                    # d2 = ||x||^2 - 2*s, rounded at the same coarse f32
                    # scale the reference uses -> matching tie behavior
                    nc.scalar.activation(
                        s_sb[:, tl, :], ps[:, :],
                        mybir.ActivationFunctionType.Identity,
                        bias=xn2_sb[:, t:t + 1], scale=-2.0)

                segmax = st_pool.tile([128, L, TPG], dt.float32, tag="segmax")
                for a, (lo, hi) in enumerate(SEGS):
                    nc.vector.tensor_reduce(
                        segmax[:, a, :], s_sb[:, :, lo:hi], axis=Ax.X,
                        op=Alu.min)

                # overwrite d2 in place with mask*(2048-iota); the max
                # over a segment is then 2048 - (first argmin index),
                # reproducing the reference's first-index tie break
                for tl in range(TPG):
                    for a, (lo, hi) in enumerate(SEGS):
                        nc.vector.scalar_tensor_tensor(
                            out=s_sb[:, tl, lo:hi],
                            in0=s_sb[:, tl, lo:hi],
                            scalar=segmax[:, a, tl:tl + 1],
                            in1=iota_sb[:, lo:hi],
                            op0=Alu.is_le,
                            op1=Alu.mult,
                        )
                segidx = st_pool.tile([128, L, TPG], dt.float32, tag="segidx")
                for a, (lo, hi) in enumerate(SEGS):
                    nc.vector.tensor_reduce(
                        segidx[:, a, :], s_sb[:, :, lo:hi], axis=Ax.X,
                        op=Alu.max)

                # running select chain over levels
                m = st_pool.tile([128, L, TPG], dt.float32, tag="m")
                win = st_pool.tile([128, TPG], dt.uint32, tag="win")
                idxf = st_pool.tile([128, L, TPG], dt.float32, tag="idxf")
                nc.vector.tensor_copy(m[:, 0, :], segmax[:, 0, :])
                nc.vector.tensor_copy(idxf[:, 0, :], segidx[:, 0, :])
                for a in range(1, L):
                    nc.vector.tensor_tensor(win[:, :], segmax[:, a, :],
                                            m[:, a - 1, :], op=Alu.is_lt)
                    nc.vector.tensor_tensor(m[:, a, :], segmax[:, a, :],
                                            m[:, a - 1, :], op=Alu.min)
                    nc.vector.tensor_copy(idxf[:, a, :], idxf[:, a - 1, :])
                    nc.vector.copy_predicated(idxf[:, a, :], win[:, :],
                                              segidx[:, a, :])
                # decode: idx = -(enc - 2048), then clamp to [0, 511]
                nc.vector.tensor_scalar(idxf[:, :, :], idxf[:, :, :],
                                        2048.0, -1.0,
                                        op0=Alu.subtract, op1=Alu.mult)
                nc.vector.tensor_scalar(idxf[:, :, :], idxf[:, :, :],
                                        float(P - 1), 0.0,
                                        op0=Alu.min, op1=Alu.max)
                idxi = st_pool.tile([128, L * TPG], dt.int32, tag="idxi")
                nc.vector.tensor_copy(idxi[:, :], idxf[:, :, :])

                # gather codebook rows: partition p, level a, tile tl gets
                # cb[idx], 64 f32 contiguous.  The DGE honors exactly one
                # dynamic offset per partition per instruction, so issue one
                # gather per (level, tile).
                gth = g_pool.tile([128, L * TPG, D], dt.float32, tag="gth")
                for pos in range(L * TPG):
                    nc.gpsimd.indirect_dma_start(
                        out=gth[:, pos, :],
                        out_offset=None,
                        in_=cb[:, :],
                        in_offset=bass.IndirectOffsetOnAxis(
                            ap=idxi[:, pos:pos + 1], axis=0),
                    )
                # line-rate output: 9KB contiguous per partition
                nc.sync.dma_start(out=out[g, :, :],
                                  in_=gth[:, :, :].rearrange("p a d -> p (a d)"))
    nc.compile()
    return nc


def _get_nc(reps=1):
    key = ("nc", reps)
    if key not in _CACHE:
        _CACHE[key] = _build_nc(reps)
    return _CACHE[key]


def _make_in_maps(inputs, codebook):
    x_full = inputs.reshape(B, D, HW)
    cT = np.empty((D + 1, P), dtype=np.float32)
    cT[:D, :] = codebook.T
    cT[D, :] = -0.5 * np.sum(codebook.astype(np.float64) ** 2,
                             axis=1).astype(np.float32)
    # descending encoding: first (lowest) index wins a max-reduce
    iota = 2048.0 - np.tile(np.arange(P, dtype=np.float32), (128, 1))
    # ||x_n||^2 per vector, per core, laid out [partition, tile]
    flat = x_full.transpose(0, 2, 1).reshape(NCORES, NLOC, D)
    xn2 = np.einsum("cnd,cnd->cn", flat, flat).astype(np.float32)
    xn2 = xn2.reshape(NCORES, TILES, 128).transpose(0, 2, 1)
    return [
        {
            "x": np.ascontiguousarray(x_full[BPC * c:BPC * (c + 1)]),
            "cT": cT,
            "cb": codebook,
            "iota": iota,
            "xn2": np.ascontiguousarray(xn2[c]),
        }
        for c in range(NCORES)
    ]


def _unstripe(raw):
    """[G, 128, L*TPG*D] -> [L, NLOC, D] with n = (g*TPG + t)*128 + p."""
    raw = raw.reshape(GROUPS, 128, L, TPG, D)
    # raw[g, p, a, t, d] -> quant[a, g, t, p, d]
    return np.ascontiguousarray(
        raw.transpose(2, 0, 3, 1, 4).reshape(L, NLOC, D))


def kernel(inputs, codebook, prev_vecs, num_vectors):
    from concourse.bass_utils import run_bass_kernel_spmd

    inputs = np.asarray(inputs, dtype=np.float32)
    codebook = np.ascontiguousarray(np.asarray(codebook, dtype=np.float32))
    assert int(num_vectors) == P
    assert inputs.shape == (B, D, 64, 64)

    nc = _get_nc()
    in_maps = _make_in_maps(inputs, codebook)
    res = run_bass_kernel_spmd(nc, in_maps, core_ids=list(range(NCORES)))
    # per-core [L, NLOC, D] -> full [L, N, D] -> [L, B, D, H, W]
    per_core = [_unstripe(res.results[c]["out"]) for c in range(NCORES)]
    quant = np.concatenate(per_core, axis=1)          # [L, B*HW, D]
    quant = quant.reshape(L, B, HW, D).transpose(0, 1, 3, 2)
    quant = np.ascontiguousarray(quant).reshape(L, B, D, 64, 64)
    actives = codebook[:P].copy()
    return quant, actives


# revision 17
# speedup vs baseline: 1.8729x; 1.8729x over previous
"""AdaptiveVectorQuantizer eval-forward on 8 TRN2 NeuronCores.

Reference computation (see problem): for flattened input vectors x [N=65536, 64]
and codebook c [512, 64], compute d2(n, p) = ||x_n||^2 + ||c_p||^2 - 2 x_n.c_p,
then for 9 prefix levels k = 2, 4, ..., 512 take idx_a = argmin_p<k d2 and emit
quant_a = c[idx_a] reshaped to [B, D, H, W].  Output: ([9, 16, 64, 64, 64], c).

Strategy (data parallel, no collectives):
  - Core c owns batches {2c, 2c+1}: 8192 vectors, 64 tiles of 128.
  - argmin_p d2 == argmax_p (x.c - ||c||^2/2): fold the -||c||^2/2 term into an
    extra contraction row, so scores come from one [65,128]x[65,512] matmul per
    tile (PSUM [128 vec, 512 codes]).
  - Prefix argmax via dyadic segments [0:2],[2:4],[4:8],...,[256:512]: batched
    DVE reduce_max per segment, within-segment index via scalar_tensor_tensor
    (mask = (s >= segmax), out = mask*iota, accum-sum -> index), then a 9-step
    running select chain over levels.
  - Gather c[idx] with gpsimd.indirect_dma_start (per-partition int32 offsets,
    256B codebook rows; the DGE on this runtime honors exactly one dynamic
    offset per partition per instruction) into [128, L*T, 64], then one
    line-rate output DMA per group (9KB contiguous per partition).  The host
    unshard de-stripes the [G, 128, L*T*D] device layout back to
    [L, B, D, H, W].
"""

import numpy as np

B, D, HW = 16, 64, 4096
P = 512
L = 9
NCORES = 8
BPC = B // NCORES            # batches per core
NLOC = BPC * HW              # vectors per core
TILE = 128
TILES = NLOC // TILE         # 64
TPG = 16                     # tiles per group
GROUPS = TILES // TPG        # 4
SEGS = [(0, 2), (2, 4)] + [(1 << a, 1 << (a + 1)) for a in range(2, 9)]

_CACHE = {}


def _build_nc(reps=1):
    import concourse.bass as bass
    import concourse.bacc as bacc
    import concourse.mybir as mybir
    from concourse.tile import TileContext

    dt = mybir.dt
    Alu = mybir.AluOpType
    Ax = mybir.AxisListType

    nc = bacc.Bacc("TRN2", target_bir_lowering=False, debug=False,
                   num_devices=NCORES)

    x = nc.dram_tensor("x", [BPC, D, HW], dt.float32, kind="ExternalInput")
    cT = nc.dram_tensor("cT", [D + 1, P], dt.float32, kind="ExternalInput")
    cb = nc.dram_tensor("cb", [P, D], dt.float32, kind="ExternalInput")
    iota = nc.dram_tensor("iota", [128, P], dt.float32, kind="ExternalInput")
    xn2 = nc.dram_tensor("xn2", [128, TILES], dt.float32,
                         kind="ExternalInput")
    out = nc.dram_tensor("out", [GROUPS, 128, L * TPG * D], dt.float32,
                         kind="ExternalOutput")

    with TileContext(nc) as tc:
        with (
            tc.tile_pool(name="const", bufs=1) as const_pool,
            tc.tile_pool(name="s", bufs=2) as s_pool,
            tc.tile_pool(name="psum", bufs=8, space="PSUM") as psum_pool,
            tc.tile_pool(name="stats", bufs=2) as st_pool,
            tc.tile_pool(name="g", bufs=2) as g_pool,
        ):
            # ---- constants / inputs ----
            cT_sb = const_pool.tile([D + 1, P], dt.float32)
            nc.sync.dma_start(out=cT_sb[:, :], in_=cT[:, :])
            iota_sb = const_pool.tile([128, P], dt.float32)
            nc.sync.dma_start(out=iota_sb[:, :], in_=iota[:, :])
            xn2_sb = const_pool.tile([128, TILES], dt.float32)
            nc.sync.dma_start(out=xn2_sb[:, :], in_=xn2[:, :])

            x_sb = const_pool.tile([D + 1, BPC, HW], dt.float32)
            for b in range(BPC):
                nc.sync.dma_start(out=x_sb[0:D, b, :], in_=x[b, :, :])
            nc.vector.memset(x_sb[D:D + 1, :, :], 1.0)


          # repeated executions of the whole pipeline (timing harness)

            for rep in range(reps):
             for g in range(GROUPS):
                s_sb = s_pool.tile([128, TPG, P], dt.float32, tag="s")
                for tl in range(TPG):
                    t = g * TPG + tl
                    b, off = divmod(t * TILE, HW)
                    ps = psum_pool.tile([128, P], dt.float32, tag="ps")
                    nc.tensor.matmul(ps[:, :], x_sb[:, b, off:off + TILE],
                                     cT_sb[:, :], start=True, stop=True)
                    # d2 = ||x||^2 - 2*s, rounded at the same coarse f32
                    # scale the reference uses -> matching tie behavior
                    nc.scalar.activation(
                        s_sb[:, tl, :], ps[:, :],
                        mybir.ActivationFunctionType.Identity,
                        bias=xn2_sb[:, t:t + 1], scale=-2.0)

                segmax = st_pool.tile([128, L, TPG], dt.float32, tag="segmax")
                for a, (lo, hi) in enumerate(SEGS):
                    nc.vector.tensor_reduce(
                        segmax[:, a, :], s_sb[:, :, lo:hi], axis=Ax.X,
                        op=Alu.min)

                # overwrite d2 in place with mask*(2048-iota); the max
                # over a segment is then 2048 - (first argmin index),
                # reproducing the reference's first-index tie break
                for tl in range(TPG):
                    for a, (lo, hi) in enumerate(SEGS):
                        nc.vector.scalar_tensor_tensor(
                            out=s_sb[:, tl, lo:hi],
                            in0=s_sb[:, tl, lo:hi],
                            scalar=segmax[:, a, tl:tl + 1],
                            in1=iota_sb[:, lo:hi],
                            op0=Alu.is_le,
                            op1=Alu.mult,
                        )
                segidx = st_pool.tile([128, L, TPG], dt.float32, tag="segidx")
                for a, (lo, hi) in enumerate(SEGS):
                    nc.vector.tensor_reduce(
                        segidx[:, a, :], s_sb[:, :, lo:hi], axis=Ax.X,
                        op=Alu.max)

                # running select chain over levels
                m = st_pool.tile([128, L, TPG], dt.float32, tag="m")
                win = st_pool.tile([128, TPG], dt.uint32, tag="win")
                idxf = st_pool.tile([128, L, TPG], dt.float32, tag="idxf")
                nc.vector.tensor_copy(m[:, 0, :], segmax[:, 0, :])
                nc.vector.tensor_copy(idxf[:, 0, :], segidx[:, 0, :])
                for a in range(1, L):
                    nc.vector.tensor_tensor(win[:, :], segmax[:, a, :],
                                            m[:, a - 1, :], op=Alu.is_lt)
                    nc.vector.tensor_tensor(m[:, a, :], segmax[:, a, :],
                                            m[:, a - 1, :], op=Alu.min)
                    nc.vector.tensor_copy(idxf[:, a, :], idxf[:, a - 1, :])
                    nc.vector.copy_predicated(idxf[:, a, :], win[:, :],
                                              segidx[:, a, :])
                # decode: idx = -(enc - 2048), then clamp to [0, 511]
                nc.vector.tensor_scalar(idxf[:, :, :], idxf[:, :, :],
                                        2048.0, -1.0,
                                        op0=Alu.subtract, op1=Alu.mult)
                nc.vector.tensor_scalar(idxf[:, :, :], idxf[:, :, :],
                                        float(P - 1), 0.0,
                                        op0=Alu.min, op1=Alu.max)
                idxi = st_pool.tile([128, L * TPG], dt.int32, tag="idxi")
                nc.vector.tensor_copy(idxi[:, :], idxf[:, :, :])

                # gather codebook rows: partition p, level a, tile tl gets
                # cb[idx], 64 f32 contiguous.  The DGE honors exactly one
                # dynamic offset per partition per instruction, so issue one
                # gather per (level, tile).
                gth = g_pool.tile([128, L * TPG, D], dt.float32, tag="gth")
                for pos in range(L * TPG):
                    nc.gpsimd.indirect_dma_start(
                        out=gth[:, pos, :],
                        out_offset=None,
                        in_=cb[:, :],
                        in_offset=bass.IndirectOffsetOnAxis(
                            ap=idxi[:, pos:pos + 1], axis=0),
                    )
                # line-rate output: 9KB contiguous per partition
                nc.sync.dma_start(out=out[g, :, :],
                                  in_=gth[:, :, :].rearrange("p a d -> p (a d)"))
    nc.compile()
    return nc


def _get_nc(reps=1):
    key = ("nc", reps)
    if key not in _CACHE:
        _CACHE[key] = _build_nc(reps)
    return _CACHE[key]


def _make_in_maps(inputs, codebook):
    x_full = inputs.reshape(B, D, HW)
    cT = np.empty((D + 1, P), dtype=np.float32)
    cT[:D, :] = codebook.T
    cT[D, :] = -0.5 * np.sum(codebook.astype(np.float64) ** 2,
                             axis=1).astype(np.float32)
    # descending encoding: first (lowest) index wins a max-reduce
    iota = 2048.0 - np.tile(np.arange(P, dtype=np.float32), (128, 1))
    # ||x_n||^2 per vector, per core, laid out [partition, tile]
    flat = x_full.transpose(0, 2, 1).reshape(NCORES, NLOC, D)
    xn2 = np.einsum("cnd,cnd->cn", flat, flat).astype(np.float32)
    xn2 = xn2.reshape(NCORES, TILES, 128).transpose(0, 2, 1)
    return [
        {
            "x": np.ascontiguousarray(x_full[BPC * c:BPC * (c + 1)]),
            "cT": cT,
            "cb": codebook,
            "iota": iota,
            "xn2": np.ascontiguousarray(xn2[c]),
        }
        for c in range(NCORES)
    ]


def _unstripe(raw):
    """[G, 128, L*TPG*D] -> [L, NLOC, D] with n = (g*TPG + t)*128 + p."""
    raw = raw.reshape(GROUPS, 128, L, TPG, D)
    # raw[g, p, a, t, d] -> quant[a, g, t, p, d]
    return np.ascontiguousarray(
        raw.transpose(2, 0, 3, 1, 4).reshape(L, NLOC, D))


def kernel(inputs, codebook, prev_vecs, num_vectors):
    from concourse.bass_utils import run_bass_kernel_spmd

    inputs = np.asarray(inputs, dtype=np.float32)
    codebook = np.ascontiguousarray(np.asarray(codebook, dtype=np.float32))
    assert int(num_vectors) == P
    assert inputs.shape == (B, D, 64, 64)

    nc = _get_nc()
    in_maps = _make_in_maps(inputs, codebook)
    res = run_bass_kernel_spmd(nc, in_maps, core_ids=list(range(NCORES)))
    # per-core [L, NLOC, D] -> full [L, N, D] -> [L, B, D, H, W]
    per_core = [_unstripe(res.results[c]["out"]) for c in range(NCORES)]
    quant = np.concatenate(per_core, axis=1)          # [L, B*HW, D]
    quant = quant.reshape(L, B, HW, D).transpose(0, 1, 3, 2)
    quant = np.ascontiguousarray(quant).reshape(L, B, D, 64, 64)
    actives = codebook[:P].copy()
    return quant, actives
